# revision 75
# baseline (speedup 1.0000x reference)
"""GAT message-passing layer on 8 Trainium2 NeuronCores (Bass/Tile).

Strategy (matches the sharding hint): nodes are partitioned across the 8
cores; each edge is owned by the core that owns its destination node, so the
segment softmax and the weighted scatter-sum stay core-local.

v2 layout (SWDGE-minimized):
  * phase 1 builds the replicated k|v projection table in DRAM from a
    host-pretransposed feature matrix (plain contiguous DMA loads, no
    DMA-transpose), and the local q table stays resident in SBUF.
  * the edge sweep fetches k and v together with a single non-transposed
    512B-row `dma_gather` per edge (one SWDGE descriptor stream instead of
    three, no xbar-transpose mode), all tensors edge-major.
  * per-edge q rows are never gathered: a one-hot dst-selector S is generated
    on-chip (compare dloc against an iota row), transposed on the PE, and
    used to expand the block's q rows with a matmul.  S also performs the
    numerator/denominator scatter-sum in one 136-column matmul per subchunk.
  * the epilogue (divide, residual, LN, FFN with PReLU folded into two
    weight matrices, LN) is emitted per group so it overlaps the gather
    stream of later groups.
"""

import sys

sys.path.insert(0, "/opt/trn_rl_repo")

import math
import os
from contextlib import ExitStack
from dataclasses import dataclass

import numpy as np
import ml_dtypes

import concourse.bass as bass
import concourse.bacc as bacc
import concourse.mybir as mybir
import concourse.tile as tile
from concourse._compat import with_exitstack
from concourse.bass_utils import run_bass_kernel_spmd
from concourse.library_config import mlp as mlp_lib

bf16 = ml_dtypes.bfloat16
P = 128
AF = mybir.ActivationFunctionType
OP = mybir.AluOpType
FP32 = mybir.dt.float32
BF16 = mybir.dt.bfloat16
I16 = mybir.dt.int16


@dataclass
class GATCfg:
    n_nodes: int = 50000
    n_edges: int = 640000
    feats: int = 128
    heads: int = 8
    dhead: int = 16
    dff: int = 512
    n_cores: int = 8
    grp: int = 2  # dst blocks per gather group
    wave: int = 4  # 128-edge subchunks per score/message wave
    pblk: int = 16  # feature blocks per phase-1 chunk
    debug: bool = False

    @property
    def npc(self):  # nodes per core
        return self.n_nodes // self.n_cores

    @property
    def nblk(self):  # local 128-node blocks per core
        return (self.npc + P - 1) // P

    @property
    def local_pad(self):
        return self.nblk * P

    @property
    def npad(self):  # padded global node count (k/v table rows)
        return ((self.n_nodes + P - 1) // P) * P

    @property
    def half(self):  # int16 index split point (block-aligned row offset base)
        # as large as int16 allows: a bigger lo half gives the lo-gather
        # prefetch more coverage while the hi table is still being built
        h = 255 * P
        if self.npad - h > 32768:
            h = ((self.npad // 2 + P - 1) // P) * P
        assert h < 32768 and (self.npad - h) <= 32768
        return h

    @property
    def ngrp(self):
        return (self.nblk + self.grp - 1) // self.grp


def _wrap16(idx):
    """int16 index list -> [128, n/16] SWDGE layout (16-wrap, replicated x8)."""
    idx = np.asarray(idx, np.int16)
    n = len(idx)
    assert n % 16 == 0
    return np.tile(idx.reshape(n // 16, 16).T, (8, 1)).copy()


def _prep(inputs, cfg: GATCfg):
    """Host-side graph partitioning / padding / index construction."""
    c = cfg
    feat = np.asarray(inputs["feat"], np.float32)
    src = np.asarray(inputs["src"], np.int64)
    dst = np.asarray(inputs["dst"], np.int64)

    feat_pad = np.zeros((c.npad, c.feats), np.float32)
    feat_pad[: c.n_nodes] = feat
    feat16T = np.ascontiguousarray(feat_pad.T.astype(bf16))  # [F, npad]

    # ---- per (core, block, half) edge lists ----
    core_of = dst // c.npc
    per_core = []
    for ci in range(c.n_cores):
        sel = np.nonzero(core_of == ci)[0]
        dloc = dst[sel] - ci * c.npc
        blk = dloc // P
        half = (src[sel] >= c.half).astype(np.int64)
        order = np.lexsort((dloc, half, blk))
        sel, dloc, blk, half = sel[order], dloc[order], blk[order], half[order]
        lists = {}
        for b in range(c.nblk):
            for h in range(2):
                m = (blk == b) & (half == h)
                lists[(b, h)] = (src[sel[m]], dloc[m])
        per_core.append(lists)

    # uniform sub-chunk counts across cores
    n_sub = np.zeros((c.nblk, 2), np.int64)
    for b in range(c.nblk):
        for h in range(2):
            mx = max(len(per_core[ci][(b, h)][0]) for ci in range(c.n_cores))
            n_sub[b, h] = (mx + P - 1) // P

    # ---- group structure (shared across cores) ----
    groups = []
    gsub0 = 0
    for g in range(c.ngrp):
        bs = list(range(g * c.grp, min((g + 1) * c.grp, c.nblk)))
        L_lo = int(sum(n_sub[b, 0] for b in bs)) * P
        L_hi = int(sum(n_sub[b, 1] for b in bs)) * P
        subs = []
        tot_per_block = {b: int(n_sub[b, 0] + n_sub[b, 1]) for b in bs}
        seen = {b: 0 for b in bs}
        s_idx = 0
        for h in range(2):
            for b in bs:
                for _ in range(int(n_sub[b, h])):
                    seen[b] += 1
                    subs.append(
                        dict(
                            block=b,
                            first=seen[b] == 1,
                            last=seen[b] == tot_per_block[b],
                            gsub=gsub0 + s_idx,
                        )
                    )
                    s_idx += 1
        groups.append(
            dict(bs=bs, L_lo=L_lo, L_hi=L_hi, L=L_lo + L_hi, subs=subs, gsub0=gsub0)
        )
        gsub0 += s_idx

    tot_subs = gsub0
    tot_lo = sum(g["L_lo"] for g in groups)
    tot_hi = sum(g["L_hi"] for g in groups)

    meta = dict(groups=groups, tot_subs=tot_subs, tot_lo=tot_lo, tot_hi=tot_hi)

    # ---- per-core streams ----
    per_core_streams = []
    for ci in range(c.n_cores):
        kv_lo = np.zeros(tot_lo, np.int16)
        kv_hi = np.zeros(tot_hi, np.int16)
        dloc_all = np.full((P, tot_subs), -1.0, np.float32)
        dloc_flat = np.full((1, tot_subs * P), -1.0, np.float32)
        olo = ohi = 0
        for g in groups:
            s_idx = 0
            for h in range(2):
                for b in g["bs"]:
                    s_arr, d_arr = per_core[ci][(b, h)]
                    nsub = int(n_sub[b, h])
                    npadded = nsub * P
                    rel = np.zeros(npadded, np.int16)
                    rel[: len(s_arr)] = (s_arr - (c.half if h else 0)).astype(
                        np.int16
                    )
                    if h == 0:
                        kv_lo[olo : olo + npadded] = rel
                        olo += npadded
                    else:
                        kv_hi[ohi : ohi + npadded] = rel
                        ohi += npadded
                    dl = np.full(npadded, -1.0, np.float32)
                    dl[: len(d_arr)] = (d_arr - b * P).astype(np.float32)
                    g0 = g["gsub0"] + s_idx
                    dloc_all[:, g0 : g0 + nsub] = dl.reshape(nsub, P).T
                    dloc_flat[0, g0 * P : g0 * P + npadded] = dl
                    s_idx += nsub
        feat32_loc = np.zeros((c.local_pad, c.feats), np.float32)
        feat32_loc[: c.npc] = feat[ci * c.npc : (ci + 1) * c.npc]
        feat16T_loc = np.ascontiguousarray(feat32_loc.T.astype(bf16))
        per_core_streams.append(
            dict(
                kv_idx_lo=_wrap16(kv_lo),
                kv_idx_hi=_wrap16(kv_hi),
                dloc_all=dloc_all.astype(bf16),
                dloc_flat=dloc_flat.astype(bf16),
                feat32_loc=feat32_loc,
                feat16T_loc=feat16T_loc,
            )
        )

    # ---- shared weight/constant tensors ----
    W1 = np.asarray(inputs["W1"], np.float32)
    W2 = np.asarray(inputs["W2"], np.float32)
    a = np.asarray(inputs["prelu_a"], np.float32)
    # prelu(x) = max(x,0) + a*min(x,0) = ((1+a)/2)*x + ((1-a)/2)*|x|
    nh = c.dff // P
    W2a = (
        (((1.0 + a) / 2.0)[:, None] * W2)
        .reshape(nh, P, c.feats)
        .transpose(1, 0, 2)
        .astype(bf16)
    )
    W2b = (
        (((1.0 - a) / 2.0)[:, None] * W2)
        .reshape(nh, P, c.feats)
        .transpose(1, 0, 2)
        .astype(bf16)
    )
    wk = np.asarray(inputs["Wk"], np.float32).astype(bf16)
    wv = np.asarray(inputs["Wv"], np.float32).astype(bf16)
    wkv = np.ascontiguousarray(np.hstack([wk, wv]))  # [F, 2F]
    iota_row = np.tile(np.arange(P, dtype=np.float32)[None, :], (P, 1)).astype(bf16)
    iota_col = np.arange(P, dtype=np.float32)[:, None].astype(bf16)
    ones1 = np.ones((1, P), np.float32).astype(bf16)
    shared = dict(
        feat16T=feat16T,
        wkv=wkv,
        wq=np.asarray(inputs["Wq"], np.float32).astype(bf16),
        w1=W1.astype(bf16),
        w2a=W2a,
        w2b=W2b,
        b1t=np.ascontiguousarray(
            np.asarray(inputs["b1"], np.float32).reshape(nh, P).T
        ),
        b2rep=np.tile(np.asarray(inputs["b2"], np.float32)[None, :], (P, 1)),
        grep=np.tile(np.asarray(inputs["ln1_g"], np.float32)[None, :], (P, 1)),
        brep=np.tile(np.asarray(inputs["ln1_b"], np.float32)[None, :], (P, 1)),
        iota_row=iota_row,
        iota_col=iota_col,
        ones1=ones1,
        epsc=np.tile(np.array([[1e-30, 1e-5]], np.float32), (P, 1)),
        zero_idx=np.zeros((P, 1), np.int16),
        ident=np.eye(P, dtype=np.float32).astype(bf16),
    )
    return meta, per_core_streams, shared


@with_exitstack
def _emit(ctx: ExitStack, tc: tile.TileContext, t, meta, cfg: GATCfg):
    """Emit the per-core program. `t` maps tensor name -> DRAM AP."""
    c = cfg
    nc = tc.nc
    groups = meta["groups"]
    nh = c.dff // P
    scale = 1.0 / math.sqrt(c.heads * c.dhead)

    with tc.tile_critical():
        nc.gpsimd.load_library(mlp_lib)

    # ---------- persistent pool: constants, indices, ft2 storage ----------
    keep = ctx.enter_context(tc.tile_pool(name="keep", bufs=1))

    def load_const(name, shape, dtype):
        tl = keep.tile(shape, dtype, tag=name)
        nc.sync.dma_start(tl[:], t[name][:])
        return tl

    wkv = load_const("wkv", [P, 2 * c.feats], BF16)
    wq = load_const("wq", [P, P], BF16)
    w1 = load_const("w1", [P, c.dff], BF16)
    w2a = load_const("w2a", [P, nh, c.feats], BF16)
    w2b = load_const("w2b", [P, nh, c.feats], BF16)
    b1t = load_const("b1t", [P, nh], FP32)
    b2rep = load_const("b2rep", [P, P], FP32)
    grep = load_const("grep", [P, P], FP32)
    brep = load_const("brep", [P, P], FP32)
    iota = load_const("iota_row", [P, P], BF16)
    iotac = load_const("iota_col", [P, 1], BF16)
    ones1 = load_const("ones1", [1, P], BF16)
    epsc = load_const("epsc", [P, 2], FP32)
    ident = load_const("ident", [P, P], BF16)
    kvlo = load_const("kv_idx_lo", [P, max(meta["tot_lo"] // 16, 1)], I16)
    kvhi = load_const("kv_idx_hi", [P, max(meta["tot_hi"] // 16, 1)], I16)
    dloc = load_const("dloc_all", [P, meta["tot_subs"]], BF16)

    ftden_sb = keep.tile([P, c.nblk, 136], FP32, tag="ftden_sb")
    qtab = keep.tile([P, c.nblk, c.feats], BF16, tag="qtab")
    zidx = load_const("zero_idx", [P, 1], I16)

    # dummy gather: absorbs the auto-inserted library unload/reload barrier at
    # t~0 so the first real gather is gated only by the kv-lo table writes
    gscratch = keep.tile([P, 1, 2 * c.feats], BF16, tag="gscratch")
    nc.gpsimd.dma_gather(
        gscratch[:],
        t["wkv"][:],
        zidx[:, 0:1],
        16,
        16,
        2 * c.feats,
        transpose=False,
        single_packet=False,
    )

    dram = ctx.enter_context(tc.tile_pool(name="dram", bufs=1, space="DRAM"))
    # two half tables: the lo-half gathers can start while hi is still built
    kv_lo_t = dram.tile([c.half, 2 * c.feats], BF16)
    kv_hi_t = dram.tile([c.npad - c.half, 2 * c.feats], BF16)

    # ---------- phase 1: projection tables (plain loads, batched writes) ----
    # SBUF pools stay allocated for the whole program: closing them would let
    # the edge pools reuse their memory, and that anti-dependency forces the
    # first gather to wait for ALL of phase 1 instead of just the lo table.
    nchunk = c.npad // P
    hblk = c.half // P
    prj_ft = ctx.enter_context(tc.tile_pool(name="prj_ft", bufs=2))
    prj_ftl = ctx.enter_context(tc.tile_pool(name="prj_ftl", bufs=1))
    prj_st = ctx.enter_context(tc.tile_pool(name="prj_st", bufs=2))

    GCHUNK = 1 << 30

    def gather_chunked(out_fn, in_ap, idx_tile, idx_off, n):
        for o in range(0, n, GCHUNK):
            m = min(GCHUNK, n - o)
            nc.gpsimd.dma_gather(
                out_fn(o, m),
                in_ap,
                idx_tile[:, (idx_off + o) // 16 : (idx_off + o + m) // 16],
                m,
                m,
                2 * c.feats,
                transpose=False,
                single_packet=False,
            )

    nsub_max = max(g["L"] for g in groups) // P
    olo_offs, ohi_offs = [], []
    olo = ohi = 0
    for g in groups:
        olo_offs.append(olo)
        ohi_offs.append(ohi)
        olo += g["L_lo"]
        ohi += g["L_hi"]

    # kvE/dflR pools live outside the phase-1 PSUM scope so the first few
    # groups' lo-half gathers can be issued between the lo and hi table builds
    eg_kv = ctx.enter_context(tc.tile_pool(name="eg_kv", bufs=3))
    eg_dfl = ctx.enter_context(tc.tile_pool(name="eg_dfl", bufs=3))
    PREF = 3  # lo-gather prefetch depth (= eg_kv bufs)
    kvEs, dflRs = {}, {}

    def issue_lo(gi):
        g = groups[gi]
        nsub = g["L"] // P
        kvE = eg_kv.tile([P, nsub_max, 2 * c.feats], BF16, tag="kvE")
        dflR = eg_dfl.tile([P, nsub_max, P], BF16, tag="dflR")
        nc.sync.dma_start(
            dflR[:, 0:nsub, :].rearrange("p s f -> p (s f)"),
            t["dloc_flat"][
                :, g["gsub0"] * P : g["gsub0"] * P + g["L"]
            ].to_broadcast([P, g["L"]]),
        )
        if g["L_lo"]:
            gather_chunked(
                lambda o, m: kvE[:, o // P : (o + m) // P, :],
                kv_lo_t[:],
                kvlo,
                olo_offs[gi],
                g["L_lo"],
            )
        kvEs[gi], dflRs[gi] = kvE, dflR

    with tc.tile_pool(name="prj_ps", bufs=4, space="PSUM") as prj_ps:
        def build_kv(b0, b1, table):
            for c0 in range(b0, b1, c.pblk):
                cb = min(c.pblk, b1 - c0)
                ft = prj_ft.tile([P, c.pblk * P], BF16, tag="ft")
                nc.sync.dma_start(
                    ft[:, 0 : cb * P], t["feat16T"][:, c0 * P : (c0 + cb) * P]
                )
                st = prj_st.tile([P, c.pblk, 2 * c.feats], BF16, tag="st")
                for i in range(cb):
                    ps = prj_ps.tile([P, 2 * c.feats], FP32, tag="kvps")
                    nc.tensor.matmul(
                        ps[:],
                        ft[:, i * P : (i + 1) * P],
                        wkv[:],
                        start=True,
                        stop=True,
                    )
                    # alternate cast engine so neither serializes phase 1
                    if i % 2:
                        nc.scalar.copy(st[:, i, :], ps[:])
                    else:
                        nc.vector.tensor_copy(st[:, i, :], ps[:])
                nc.sync.dma_start(
                    table[:]
                    .rearrange("(s p) f -> p s f", p=P)[:, c0 - b0 : c0 - b0 + cb, :],
                    st[:, 0:cb, :],
                )

        build_kv(0, hblk, kv_lo_t)
        # lo-half gathers of the first groups run while the hi table is built
        for gi in range(min(PREF, len(groups))):
            issue_lo(gi)
        build_kv(hblk, nchunk, kv_hi_t)
        # local q table (needed only once the first group's compute starts)
        ftl = prj_ftl.tile([P, c.nblk * P], BF16, tag="ftl")
        nc.sync.dma_start(ftl[:], t["feat16T_loc"][:])
        for b in range(c.nblk):
            ps = prj_ps.tile([P, 2 * c.feats], FP32, tag="kvps")
            nc.tensor.matmul(
                ps[:, 0:P], ftl[:, b * P : (b + 1) * P], wq[:], start=True, stop=True
            )
            nc.any.tensor_copy(qtab[:, b, :], ps[:, 0:P])

    # ---------- phase 2+3: edge sweep with inline epilogue ----------
    def layernorm(pool, x32, nb, out_dtype=FP32):
        """x32: [P, nb, 128] fp32 SBUF tile -> normalized * g + b (new tile)."""
        msum = pool.tile([P, c.grp], FP32, tag="ln_msum")
        nc.vector.tensor_reduce(
            msum[:, 0:nb], x32[:, 0:nb, :], axis=mybir.AxisListType.X, op=OP.add
        )
        # scalar steps run on ACT: DVE tensor_scalar enters 2-port perf mode
        # and collides with SWDGE descriptor generation on GpSimd
        nmean = pool.tile([P, c.grp], FP32, tag="ln_nmean")
        nc.scalar.activation(
            nmean[:, 0:nb], msum[:, 0:nb], AF.Identity, scale=-1.0 / c.feats
        )
        sq = pool.tile([P, c.grp, P], FP32, tag="ln_sq")
        for b in range(nb):
            nc.scalar.activation(
                sq[:, b],
                x32[:, b],
                AF.Square,
                bias=nmean[:, b : b + 1],
            )
        var = pool.tile([P, c.grp], FP32, tag="ln_var")
        nc.vector.tensor_reduce(
            var[:, 0:nb], sq[:, 0:nb, :], axis=mybir.AxisListType.X, op=OP.add
        )
        rstd = pool.tile([P, c.grp], FP32, tag="ln_rstd")
        nc.scalar.activation(
            rstd[:, 0:nb],
            var[:, 0:nb],
            AF.Identity,
            scale=1.0 / c.feats,
            bias=epsc[:, 1:2],
        )
        nc.vector.reciprocal(rstd[:, 0:nb], rstd[:, 0:nb])
        nc.scalar.sqrt(rstd[:, 0:nb], rstd[:, 0:nb])
        nmr = pool.tile([P, c.grp], FP32, tag="ln_nmr")
        nc.vector.tensor_tensor(
            nmr[:, 0:nb], nmean[:, 0:nb], rstd[:, 0:nb], op=OP.mult
        )
        normed = pool.tile([P, c.grp, P], FP32, tag="ln_normed")
        for b in range(nb):
            nc.scalar.activation(
                normed[:, b],
                x32[:, b],
                AF.Identity,
                scale=rstd[:, b : b + 1],
                bias=nmr[:, b : b + 1],
            )
        out = pool.tile([P, c.grp, P], out_dtype, tag="ln_out" + str(out_dtype))
        nc.vector.tensor_tensor(
            out[:, 0:nb],
            normed[:, 0:nb],
            grep[:].rearrange("p (o f) -> p o f", o=1).to_broadcast([P, nb, P]),
            op=OP.mult,
        )
        nc.vector.tensor_tensor(
            out[:, 0:nb],
            out[:, 0:nb],
            brep[:].rearrange("p (o f) -> p o f", o=1).to_broadcast([P, nb, P]),
            op=OP.add,
        )
        return out

    with (
        tc.tile_pool(name="eg_wv", bufs=2) as eg_wv,
        tc.tile_pool(name="eg_ps", bufs=3, space="PSUM") as eg_ps,
        tc.tile_pool(name="eg_ftps", bufs=c.grp + 1, space="PSUM") as eg_ftps,
        tc.tile_pool(name="ep", bufs=2) as ep,
        tc.tile_pool(name="ep_ps", bufs=1, space="PSUM") as ep_ps,
        tc.tile_pool(name="ep_h1ps", bufs=1, space="PSUM") as ep_h1ps,
    ):
        for gi, g in enumerate(groups):
            L, L_lo, L_hi = g["L"], g["L_lo"], g["L_hi"]
            nsub = L // P
            kvE, dflR = kvEs.pop(gi), dflRs.pop(gi)
            if L_hi:
                gather_chunked(
                    lambda o, m: kvE[:, (L_lo + o) // P : (L_lo + o + m) // P, :],
                    kv_hi_t[:],
                    kvhi,
                    ohi_offs[gi],
                    L_hi,
                )

            subs = g["subs"]
            g0 = g["gsub0"]
            # one-hot dst selectors for the whole group (one DVE op each):
            # S[e, dst] from per-partition dloc, S^T[dst, e] from replicated dloc
            Sw = eg_wv.tile([P, nsub_max, P], BF16, tag="Sw")
            nc.vector.tensor_tensor(
                Sw[:, 0:nsub],
                dloc[:, g0 : g0 + nsub]
                .rearrange("p (w o) -> p w o", o=1)
                .to_broadcast([P, nsub, P]),
                iota[:].rearrange("p (o f) -> p o f", o=1).to_broadcast(
                    [P, nsub, P]
                ),
                op=OP.is_equal,
            )
            # S^T in place over the replicated dloc values
            nc.vector.tensor_tensor(
                dflR[:, 0:nsub],
                dflR[:, 0:nsub],
                iotac[:]
                .rearrange("p (w o) -> p w o", o=1)
                .to_broadcast([P, nsub, P]),
                op=OP.is_equal,
            )
            # per-wave: qE = S^T.T @ q_blk (PSUM), TT = kE * qE
            # TT and Mt share one [*, 136] tile: TT lives in cols 0:128 until
            # the head-reduce, then the same cols are overwritten with v*exp
            TM = eg_wv.tile([P, nsub_max, 136], BF16, tag="TM")
            for w0 in range(0, len(subs), c.wave):
                wsubs = subs[w0 : w0 + c.wave]
                wl = len(wsubs)
                qeps = eg_ps.tile([P, c.wave * P], FP32, tag="b32", name="qeps")
                for i, s in enumerate(wsubs):
                    nc.tensor.matmul(
                        qeps[:, i * P : (i + 1) * P],
                        dflR[:, w0 + i, :],
                        qtab[:, s["block"], :],
                        start=True,
                        stop=True,
                    )
                nc.vector.tensor_tensor(
                    TM[:, w0 : w0 + wl, 0:128],
                    kvE[:, w0 : w0 + wl, 0 : c.feats],
                    qeps[:, 0 : wl * P].rearrange("p (w f) -> p w f", f=P),
                    op=OP.mult,
                )
            # group-level: head-reduce, exp, weighted messages
            sc = eg_wv.tile([P, nsub_max, c.heads], FP32, tag="sc")
            nc.vector.tensor_reduce(
                sc[:, 0:nsub],
                TM[:, 0:nsub, 0:128].rearrange("p w (h d) -> p w h d", d=c.dhead),
                axis=mybir.AxisListType.X,
                op=OP.add,
            )
            nc.scalar.activation(
                TM[:, 0:nsub, 128:136], sc[:, 0:nsub], AF.Exp, scale=scale
            )
            nc.vector.tensor_tensor(
                TM[:, 0:nsub, 0:128].rearrange("p w (h d) -> p w h d", d=c.dhead),
                kvE[:, 0:nsub, c.feats : 2 * c.feats].rearrange(
                    "p w (h d) -> p w h d", d=c.dhead
                ),
                TM[:, 0:nsub, 128:136]
                .rearrange("p w (h o) -> p w h o", o=1)
                .to_broadcast([P, nsub, c.heads, c.dhead]),
                op=OP.mult,
            )
            ftps = {}
            for idx, s in enumerate(subs):
                b = s["block"]
                if s["first"]:
                    ftps[b] = eg_ftps.tile([P, 136], FP32, tag="ftps", name="ftps")
                nc.tensor.matmul(
                    ftps[b][:],
                    Sw[:, idx, :],
                    TM[:, idx, :],
                    start=s["first"],
                    stop=s["last"],
                    skip_group_check=True,
                )
                if s["last"]:
                    # ACT copy: a DVE copy could enter 2-port mode and block
                    # the concurrent SWDGE gather emission on GpSimd
                    nc.scalar.copy(ftden_sb[:, b, :], ftps[b][:])
                    del ftps[b]

            # ---------- epilogue for this group ----------
            bs = g["bs"]
            nb = len(bs)
            b0 = bs[0]
            f32 = ep.tile([P, c.grp, P], FP32, tag="f32")
            nc.sync.dma_start(
                f32[:, 0:nb, :],
                t["feat32_loc"][:]
                .rearrange("(s p) f -> p s f", p=P)[:, b0 : b0 + nb, :],
            )
            r = ep.tile([P, c.grp, c.heads], FP32, tag="recip")
            nc.scalar.activation(
                r[:, 0:nb], ftden_sb[:, b0 : b0 + nb, 128:136], AF.Identity,
                bias=epsc[:, 0:1],
            )
            nc.vector.reciprocal(r[:, 0:nb], r[:, 0:nb])
            rst = ep.tile([P, c.grp, P], FP32, tag="rst")
            nc.vector.tensor_tensor(
                rst[:, 0:nb],
                ftden_sb[:, b0 : b0 + nb, 0:128].rearrange(
                    "p s (h d) -> p s h d", d=c.dhead
                ),
                r[:, 0:nb].rearrange("p s (h o) -> p s h o", o=1).to_broadcast(
                    [P, nb, c.heads, c.dhead]
                ),
                op=OP.mult,
            )
            nc.vector.tensor_tensor(
                rst[:, 0:nb], rst[:, 0:nb], f32[:, 0:nb, :], op=OP.add
            )
            ln1 = layernorm(ep, rst, nb)
            ln1b = ep.tile([P, c.grp, P], BF16, tag="ln1b")
            nc.scalar.copy(ln1b[:, 0:nb], ln1[:, 0:nb])
            # transpose ln1 -> feat-major for FFN (rT and ffps share one bank):
            # bf16 transposes in cols 0:grp*P, fp32 ffps behind via bitcast
            ep2 = ep_ps.tile([P, 4 * c.grp * P], BF16, tag="ep2")
            for b in range(nb):
                nc.tensor.transpose(
                    ep2[:, b * P : (b + 1) * P], ln1b[:, b, :], ident[:]
                )
            rT = ep.tile([P, c.grp * P], BF16, tag="rT")
            nc.scalar.copy(rT[:, 0 : nb * P], ep2[:, 0 : nb * P])
            # H1 = W1.T @ rT  (feat-major, nh slices) ; prelu via W2a/W2b trick
            ff0 = 2 * c.grp * P  # ffps fp32 region starts here (in bf16 cols)
            for h in range(nh):
                h1ps = ep_h1ps.tile([P, c.grp * P], FP32, tag="h1ps")
                nc.tensor.matmul(
                    h1ps[:, 0 : nb * P],
                    w1[:, h * P : (h + 1) * P],
                    rT[:, 0 : nb * P],
                    start=True,
                    stop=True,
                )
                h1sb = ep.tile([P, c.grp * P], BF16, tag="h1sb")
                nc.scalar.activation(
                    h1sb[:, 0 : nb * P],
                    h1ps[:, 0 : nb * P],
                    AF.Identity,
                    bias=b1t[:, h : h + 1],
                )
                habs = ep.tile([P, c.grp * P], BF16, tag="habs")
                nc.scalar.activation(
                    habs[:, 0 : nb * P],
                    h1ps[:, 0 : nb * P],
                    AF.Abs,
                    bias=b1t[:, h : h + 1],
                )
                for b in range(nb):
                    ffb = ep2[:, ff0 + 2 * b * P : ff0 + 2 * (b + 1) * P].bitcast(
                        FP32
                    )
                    nc.tensor.matmul(
                        ffb,
                        h1sb[:, b * P : (b + 1) * P],
                        w2a[:, h, :],
                        start=(h == 0 and b == 0),
                        stop=False,
                        skip_group_check=True,
                    )
                    nc.tensor.matmul(
                        ffb,
                        habs[:, b * P : (b + 1) * P],
                        w2b[:, h, :],
                        start=False,
                        stop=(h == nh - 1),
                        skip_group_check=True,
                    )
            rst2 = ep.tile([P, c.grp, P], FP32, tag="rst2")
            nc.vector.tensor_tensor(
                rst2[:, 0:nb],
                ep2[:, ff0 : ff0 + 2 * nb * P]
                .bitcast(FP32)
                .rearrange("p (s f) -> p s f", f=P),
                ln1[:, 0:nb],
                op=OP.add,
            )
            nc.vector.tensor_tensor(
                rst2[:, 0:nb],
                rst2[:, 0:nb],
                b2rep[:].rearrange("p (o f) -> p o f", o=1).to_broadcast([P, nb, P]),
                op=OP.add,
            )
            ln2 = layernorm(ep, rst2, nb)
            nc.sync.dma_start(
                t["out"][:].rearrange("(s p) f -> p s f", p=P)[:, b0 : b0 + nb, :],
                ln2[:, 0:nb],
            )
            if gi + PREF < len(groups):
                issue_lo(gi + PREF)


def _build(meta, cfg: GATCfg):
    c = cfg
    nc = bacc.Bacc("TRN2", target_bir_lowering=False, debug=False, num_devices=c.n_cores)
    t = {}

    def inp(name, shape, dtype):
        t[name] = nc.dram_tensor(name, shape, dtype, kind="ExternalInput").ap()

    inp("feat16T", [c.feats, c.npad], BF16)
    inp("feat16T_loc", [c.feats, c.local_pad], BF16)
    inp("feat32_loc", [c.local_pad, c.feats], FP32)
    inp("wkv", [c.feats, 2 * c.feats], BF16)
    inp("wq", [c.feats, c.feats], BF16)
    inp("w1", [c.feats, c.dff], BF16)
    inp("w2a", [P, c.dff // P, c.feats], BF16)
    inp("w2b", [P, c.dff // P, c.feats], BF16)
    inp("b1t", [P, c.dff // P], FP32)
    inp("b2rep", [P, c.feats], FP32)
    inp("grep", [P, c.feats], FP32)
    inp("brep", [P, c.feats], FP32)
    inp("iota_row", [P, P], BF16)
    inp("iota_col", [P, 1], BF16)
    inp("ones1", [1, P], BF16)
    inp("epsc", [P, 2], FP32)
    inp("zero_idx", [P, 1], I16)
    inp("ident", [P, P], BF16)
    inp("kv_idx_lo", [P, max(meta["tot_lo"] // 16, 1)], I16)
    inp("kv_idx_hi", [P, max(meta["tot_hi"] // 16, 1)], I16)
    inp("dloc_all", [P, meta["tot_subs"]], BF16)
    inp("dloc_flat", [1, meta["tot_subs"] * P], BF16)
    t["out"] = nc.dram_tensor(
        "out", [c.local_pad, c.feats], FP32, kind="ExternalOutput"
    ).ap()

    with tile.TileContext(nc) as tc:
        _emit(tc, t, meta, cfg)
    nc.compile()
    return nc


def _in_maps(meta, streams, shared, cfg: GATCfg):
    maps = []
    for ci in range(cfg.n_cores):
        m = dict(shared)
        st = streams[ci]
        m["feat16T_loc"] = st["feat16T_loc"]
        m["feat32_loc"] = st["feat32_loc"]
        m["kv_idx_lo"] = (
            st["kv_idx_lo"] if meta["tot_lo"] else np.zeros((P, 1), np.int16)
        )
        m["kv_idx_hi"] = (
            st["kv_idx_hi"] if meta["tot_hi"] else np.zeros((P, 1), np.int16)
        )
        m["dloc_all"] = st["dloc_all"]
        m["dloc_flat"] = st["dloc_flat"]
        maps.append(m)
    return maps


_CACHE = {}


def kernel(**inputs) -> np.ndarray:
    cfg = GATCfg()
    meta, streams, shared = _prep(inputs, cfg)
    key = "real"
    if key not in _CACHE:
        _CACHE[key] = _build(meta, cfg)
    nc = _CACHE[key]
    maps = _in_maps(meta, streams, shared, cfg)
    res = run_bass_kernel_spmd(nc, maps, core_ids=list(range(cfg.n_cores)))
    out = np.empty((cfg.n_nodes, cfg.feats), np.float32)
    for ci in range(cfg.n_cores):
        out[ci * cfg.npc : (ci + 1) * cfg.npc] = res.results[ci]["out"][: cfg.npc]
    return out


# revision 76
# speedup vs baseline: 1.0848x; 1.0848x over previous
"""GAT message-passing layer on 8 Trainium2 NeuronCores (Bass/Tile).

Strategy (matches the sharding hint): nodes are partitioned across the 8
cores; each edge is owned by the core that owns its destination node, so the
segment softmax and the weighted scatter-sum stay core-local.

v2 layout (SWDGE-minimized):
  * phase 1 builds the replicated k|v projection table in DRAM from a
    host-pretransposed feature matrix (plain contiguous DMA loads, no
    DMA-transpose), and the local q table stays resident in SBUF.
  * the edge sweep fetches k and v together with a single non-transposed
    512B-row `dma_gather` per edge (one SWDGE descriptor stream instead of
    three, no xbar-transpose mode), all tensors edge-major.
  * per-edge q rows are never gathered: a one-hot dst-selector S is generated
    on-chip (compare dloc against an iota row), transposed on the PE, and
    used to expand the block's q rows with a matmul.  S also performs the
    numerator/denominator scatter-sum in one 136-column matmul per subchunk.
  * the epilogue (divide, residual, LN, FFN with PReLU folded into two
    weight matrices, LN) is emitted per group so it overlaps the gather
    stream of later groups.
"""

import sys

sys.path.insert(0, "/opt/trn_rl_repo")

import math
import os
from contextlib import ExitStack
from dataclasses import dataclass

import numpy as np
import ml_dtypes

import concourse.bass as bass
import concourse.bacc as bacc
import concourse.mybir as mybir
import concourse.tile as tile
from concourse._compat import with_exitstack
from concourse.bass_utils import run_bass_kernel_spmd
from concourse.library_config import mlp as mlp_lib

bf16 = ml_dtypes.bfloat16
P = 128
AF = mybir.ActivationFunctionType
OP = mybir.AluOpType
FP32 = mybir.dt.float32
BF16 = mybir.dt.bfloat16
I16 = mybir.dt.int16


@dataclass
class GATCfg:
    n_nodes: int = 50000
    n_edges: int = 640000
    feats: int = 128
    heads: int = 8
    dhead: int = 16
    dff: int = 512
    n_cores: int = 8
    grp: int = 2  # dst blocks per gather group
    wave: int = 4  # 128-edge subchunks per score/message wave
    pblk: int = 16  # feature blocks per phase-1 chunk
    debug: bool = False

    @property
    def npc(self):  # nodes per core
        return self.n_nodes // self.n_cores

    @property
    def nblk(self):  # local 128-node blocks per core
        return (self.npc + P - 1) // P

    @property
    def local_pad(self):
        return self.nblk * P

    @property
    def npad(self):  # padded global node count (k/v table rows)
        return ((self.n_nodes + P - 1) // P) * P

    @property
    def half(self):  # int16 index split point (block-aligned row offset base)
        h = ((self.npad // 2 + P - 1) // P) * P
        assert h < 32768 and (self.npad - h) <= 32768
        return h

    @property
    def ngrp(self):
        return (self.nblk + self.grp - 1) // self.grp


def _wrap16(idx):
    """int16 index list -> [128, n/16] SWDGE layout (16-wrap, replicated x8)."""
    idx = np.asarray(idx, np.int16)
    n = len(idx)
    assert n % 16 == 0
    return np.tile(idx.reshape(n // 16, 16).T, (8, 1)).copy()


def _prep(inputs, cfg: GATCfg):
    """Host-side graph partitioning / padding / index construction."""
    c = cfg
    feat = np.asarray(inputs["feat"], np.float32)
    src = np.asarray(inputs["src"], np.int64)
    dst = np.asarray(inputs["dst"], np.int64)

    feat_pad = np.zeros((c.npad, c.feats), np.float32)
    feat_pad[: c.n_nodes] = feat
    feat16T = np.ascontiguousarray(feat_pad.T.astype(bf16))  # [F, npad]

    # ---- per (core, block, half) edge lists ----
    core_of = dst // c.npc
    per_core = []
    for ci in range(c.n_cores):
        sel = np.nonzero(core_of == ci)[0]
        dloc = dst[sel] - ci * c.npc
        blk = dloc // P
        half = (src[sel] >= c.half).astype(np.int64)
        order = np.lexsort((dloc, half, blk))
        sel, dloc, blk, half = sel[order], dloc[order], blk[order], half[order]
        lists = {}
        for b in range(c.nblk):
            for h in range(2):
                m = (blk == b) & (half == h)
                lists[(b, h)] = (src[sel[m]], dloc[m])
        per_core.append(lists)

    # uniform sub-chunk counts across cores
    n_sub = np.zeros((c.nblk, 2), np.int64)
    for b in range(c.nblk):
        for h in range(2):
            mx = max(len(per_core[ci][(b, h)][0]) for ci in range(c.n_cores))
            n_sub[b, h] = (mx + P - 1) // P

    # ---- group structure (shared across cores) ----
    groups = []
    gsub0 = 0
    for g in range(c.ngrp):
        bs = list(range(g * c.grp, min((g + 1) * c.grp, c.nblk)))
        L_lo = int(sum(n_sub[b, 0] for b in bs)) * P
        L_hi = int(sum(n_sub[b, 1] for b in bs)) * P
        subs = []
        tot_per_block = {b: int(n_sub[b, 0] + n_sub[b, 1]) for b in bs}
        seen = {b: 0 for b in bs}
        s_idx = 0
        for h in range(2):
            for b in bs:
                for _ in range(int(n_sub[b, h])):
                    seen[b] += 1
                    subs.append(
                        dict(
                            block=b,
                            first=seen[b] == 1,
                            last=seen[b] == tot_per_block[b],
                            gsub=gsub0 + s_idx,
                        )
                    )
                    s_idx += 1
        groups.append(
            dict(bs=bs, L_lo=L_lo, L_hi=L_hi, L=L_lo + L_hi, subs=subs, gsub0=gsub0)
        )
        gsub0 += s_idx

    tot_subs = gsub0
    tot_lo = sum(g["L_lo"] for g in groups)
    tot_hi = sum(g["L_hi"] for g in groups)

    meta = dict(groups=groups, tot_subs=tot_subs, tot_lo=tot_lo, tot_hi=tot_hi)

    # ---- per-core streams ----
    per_core_streams = []
    for ci in range(c.n_cores):
        kv_lo = np.zeros(tot_lo, np.int16)
        kv_hi = np.zeros(tot_hi, np.int16)
        dloc_all = np.full((P, tot_subs), -1.0, np.float32)
        dloc_flat = np.full((1, tot_subs * P), -1.0, np.float32)
        olo = ohi = 0
        for g in groups:
            s_idx = 0
            for h in range(2):
                for b in g["bs"]:
                    s_arr, d_arr = per_core[ci][(b, h)]
                    nsub = int(n_sub[b, h])
                    npadded = nsub * P
                    rel = np.zeros(npadded, np.int16)
                    rel[: len(s_arr)] = (s_arr - (c.half if h else 0)).astype(
                        np.int16
                    )
                    if h == 0:
                        kv_lo[olo : olo + npadded] = rel
                        olo += npadded
                    else:
                        kv_hi[ohi : ohi + npadded] = rel
                        ohi += npadded
                    dl = np.full(npadded, -1.0, np.float32)
                    dl[: len(d_arr)] = (d_arr - b * P).astype(np.float32)
                    g0 = g["gsub0"] + s_idx
                    dloc_all[:, g0 : g0 + nsub] = dl.reshape(nsub, P).T
                    dloc_flat[0, g0 * P : g0 * P + npadded] = dl
                    s_idx += nsub
        feat32_loc = np.zeros((c.local_pad, c.feats), np.float32)
        feat32_loc[: c.npc] = feat[ci * c.npc : (ci + 1) * c.npc]
        feat16T_loc = np.ascontiguousarray(feat32_loc.T.astype(bf16))
        per_core_streams.append(
            dict(
                kv_idx_lo=_wrap16(kv_lo),
                kv_idx_hi=_wrap16(kv_hi),
                dloc_all=dloc_all.astype(bf16),
                dloc_flat=dloc_flat.astype(bf16),
                feat32_loc=feat32_loc,
                feat16T_loc=feat16T_loc,
            )
        )

    # ---- shared weight/constant tensors ----
    W1 = np.asarray(inputs["W1"], np.float32)
    W2 = np.asarray(inputs["W2"], np.float32)
    a = np.asarray(inputs["prelu_a"], np.float32)
    # prelu(x) = max(x,0) + a*min(x,0) = ((1+a)/2)*x + ((1-a)/2)*|x|
    nh = c.dff // P
    W2a = (
        (((1.0 + a) / 2.0)[:, None] * W2)
        .reshape(nh, P, c.feats)
        .transpose(1, 0, 2)
        .astype(bf16)
    )
    W2b = (
        (((1.0 - a) / 2.0)[:, None] * W2)
        .reshape(nh, P, c.feats)
        .transpose(1, 0, 2)
        .astype(bf16)
    )
    wk = np.asarray(inputs["Wk"], np.float32).astype(bf16)
    wv = np.asarray(inputs["Wv"], np.float32).astype(bf16)
    wkv = np.ascontiguousarray(np.hstack([wk, wv]))  # [F, 2F]
    iota_row = np.tile(np.arange(P, dtype=np.float32)[None, :], (P, 1)).astype(bf16)
    iota_col = np.arange(P, dtype=np.float32)[:, None].astype(bf16)
    ones1 = np.ones((1, P), np.float32).astype(bf16)
    shared = dict(
        feat16T=feat16T,
        wkv=wkv,
        wq=np.asarray(inputs["Wq"], np.float32).astype(bf16),
        w1=W1.astype(bf16),
        w2a=W2a,
        w2b=W2b,
        b1t=np.ascontiguousarray(
            np.asarray(inputs["b1"], np.float32).reshape(nh, P).T
        ),
        b2rep=np.tile(np.asarray(inputs["b2"], np.float32)[None, :], (P, 1)),
        grep=np.tile(np.asarray(inputs["ln1_g"], np.float32)[None, :], (P, 1)),
        brep=np.tile(np.asarray(inputs["ln1_b"], np.float32)[None, :], (P, 1)),
        iota_row=iota_row,
        iota_col=iota_col,
        ones1=ones1,
        epsc=np.tile(np.array([[1e-30, 1e-5]], np.float32), (P, 1)),
        zero_idx=np.zeros((P, 1), np.int16),
        ident=np.eye(P, dtype=np.float32).astype(bf16),
    )
    return meta, per_core_streams, shared


@with_exitstack
def _emit(ctx: ExitStack, tc: tile.TileContext, t, meta, cfg: GATCfg):
    """Emit the per-core program. `t` maps tensor name -> DRAM AP."""
    c = cfg
    nc = tc.nc
    groups = meta["groups"]
    nh = c.dff // P
    scale = 1.0 / math.sqrt(c.heads * c.dhead)

    with tc.tile_critical():
        nc.gpsimd.load_library(mlp_lib)

    # ---------- persistent pool: constants, indices, ft2 storage ----------
    keep = ctx.enter_context(tc.tile_pool(name="keep", bufs=1))

    def load_const(name, shape, dtype):
        tl = keep.tile(shape, dtype, tag=name)
        nc.sync.dma_start(tl[:], t[name][:])
        return tl

    wkv = load_const("wkv", [P, 2 * c.feats], BF16)
    wq = load_const("wq", [P, P], BF16)
    w1 = load_const("w1", [P, c.dff], BF16)
    w2a = load_const("w2a", [P, nh, c.feats], BF16)
    w2b = load_const("w2b", [P, nh, c.feats], BF16)
    b1t = load_const("b1t", [P, nh], FP32)
    b2rep = load_const("b2rep", [P, P], FP32)
    grep = load_const("grep", [P, P], FP32)
    brep = load_const("brep", [P, P], FP32)
    iota = load_const("iota_row", [P, P], BF16)
    iotac = load_const("iota_col", [P, 1], BF16)
    ones1 = load_const("ones1", [1, P], BF16)
    epsc = load_const("epsc", [P, 2], FP32)
    ident = load_const("ident", [P, P], BF16)
    kvlo = load_const("kv_idx_lo", [P, max(meta["tot_lo"] // 16, 1)], I16)
    kvhi = load_const("kv_idx_hi", [P, max(meta["tot_hi"] // 16, 1)], I16)
    dloc = load_const("dloc_all", [P, meta["tot_subs"]], BF16)

    ftden_sb = keep.tile([P, c.nblk, 136], FP32, tag="ftden_sb")
    qtab = keep.tile([P, c.nblk, c.feats], BF16, tag="qtab")
    zidx = load_const("zero_idx", [P, 1], I16)

    # dummy gather: absorbs the auto-inserted library unload/reload barrier at
    # t~0 so the first real gather is gated only by the kv-lo table writes
    gscratch = keep.tile([P, 1, 2 * c.feats], BF16, tag="gscratch")
    nc.gpsimd.dma_gather(
        gscratch[:],
        t["wkv"][:],
        zidx[:, 0:1],
        16,
        16,
        2 * c.feats,
        transpose=False,
        single_packet=False,
    )

    dram = ctx.enter_context(tc.tile_pool(name="dram", bufs=1, space="DRAM"))
    # two half tables: the lo-half gathers can start while hi is still built
    kv_lo_t = dram.tile([c.half, 2 * c.feats], BF16)
    kv_hi_t = dram.tile([c.npad - c.half, 2 * c.feats], BF16)

    # ---------- phase 1: projection tables (plain loads, batched writes) ----
    # SBUF pools stay allocated for the whole program: closing them would let
    # the edge pools reuse their memory, and that anti-dependency forces the
    # first gather to wait for ALL of phase 1 instead of just the lo table.
    nchunk = c.npad // P
    hblk = c.half // P
    prj_ft = ctx.enter_context(tc.tile_pool(name="prj_ft", bufs=2))
    prj_ftl = ctx.enter_context(tc.tile_pool(name="prj_ftl", bufs=1))
    prj_st = ctx.enter_context(tc.tile_pool(name="prj_st", bufs=2))

    GCHUNK = 1 << 30

    def gather_chunked(out_fn, in_ap, idx_tile, idx_off, n):
        for o in range(0, n, GCHUNK):
            m = min(GCHUNK, n - o)
            nc.gpsimd.dma_gather(
                out_fn(o, m),
                in_ap,
                idx_tile[:, (idx_off + o) // 16 : (idx_off + o + m) // 16],
                m,
                m,
                2 * c.feats,
                transpose=False,
                single_packet=False,
            )

    nsub_max = max(g["L"] for g in groups) // P
    olo_offs, ohi_offs = [], []
    olo = ohi = 0
    for g in groups:
        olo_offs.append(olo)
        ohi_offs.append(ohi)
        olo += g["L_lo"]
        ohi += g["L_hi"]

    # kvE/dflR pools live outside the phase-1 PSUM scope so the first few
    # groups' lo-half gathers can be issued between the lo and hi table builds
    eg_kv = ctx.enter_context(tc.tile_pool(name="eg_kv", bufs=3))
    eg_dfl = ctx.enter_context(tc.tile_pool(name="eg_dfl", bufs=3))
    PREF = 3  # lo-gather prefetch depth (= eg_kv bufs)
    kvEs, dflRs = {}, {}

    def issue_lo(gi):
        g = groups[gi]
        nsub = g["L"] // P
        kvE = eg_kv.tile([P, nsub_max, 2 * c.feats], BF16, tag="kvE")
        dflR = eg_dfl.tile([P, nsub_max, P], BF16, tag="dflR")
        nc.sync.dma_start(
            dflR[:, 0:nsub, :].rearrange("p s f -> p (s f)"),
            t["dloc_flat"][
                :, g["gsub0"] * P : g["gsub0"] * P + g["L"]
            ].to_broadcast([P, g["L"]]),
        )
        if g["L_lo"]:
            gather_chunked(
                lambda o, m: kvE[:, o // P : (o + m) // P, :],
                kv_lo_t[:],
                kvlo,
                olo_offs[gi],
                g["L_lo"],
            )
        kvEs[gi], dflRs[gi] = kvE, dflR

    with tc.tile_pool(name="prj_ps", bufs=4, space="PSUM") as prj_ps:
        def build_kv(b0, b1, table):
            for c0 in range(b0, b1, c.pblk):
                cb = min(c.pblk, b1 - c0)
                ft = prj_ft.tile([P, c.pblk * P], BF16, tag="ft")
                nc.sync.dma_start(
                    ft[:, 0 : cb * P], t["feat16T"][:, c0 * P : (c0 + cb) * P]
                )
                st = prj_st.tile([P, c.pblk, 2 * c.feats], BF16, tag="st")
                for i in range(cb):
                    ps = prj_ps.tile([P, 2 * c.feats], FP32, tag="kvps")
                    nc.tensor.matmul(
                        ps[:],
                        ft[:, i * P : (i + 1) * P],
                        wkv[:],
                        start=True,
                        stop=True,
                    )
                    # alternate cast engine so neither serializes phase 1
                    if i % 2:
                        nc.scalar.copy(st[:, i, :], ps[:])
                    else:
                        nc.vector.tensor_copy(st[:, i, :], ps[:])
                nc.sync.dma_start(
                    table[:]
                    .rearrange("(s p) f -> p s f", p=P)[:, c0 - b0 : c0 - b0 + cb, :],
                    st[:, 0:cb, :],
                )

        build_kv(0, hblk, kv_lo_t)
        # lo-half gathers of the first groups run while the hi table is built
        for gi in range(min(PREF, len(groups))):
            issue_lo(gi)
        build_kv(hblk, nchunk, kv_hi_t)
        # local q table (needed only once the first group's compute starts)
        ftl = prj_ftl.tile([P, c.nblk * P], BF16, tag="ftl")
        nc.sync.dma_start(ftl[:], t["feat16T_loc"][:])
        for b in range(c.nblk):
            ps = prj_ps.tile([P, 2 * c.feats], FP32, tag="kvps")
            nc.tensor.matmul(
                ps[:, 0:P], ftl[:, b * P : (b + 1) * P], wq[:], start=True, stop=True
            )
            nc.any.tensor_copy(qtab[:, b, :], ps[:, 0:P])

    # ---------- phase 2+3: edge sweep with inline epilogue ----------
    def layernorm(pool, x32, nb, out_dtype=FP32):
        """x32: [P, nb, 128] fp32 SBUF tile -> normalized * g + b (new tile)."""
        msum = pool.tile([P, c.grp], FP32, tag="ln_msum")
        nc.vector.tensor_reduce(
            msum[:, 0:nb], x32[:, 0:nb, :], axis=mybir.AxisListType.X, op=OP.add
        )
        # scalar steps run on ACT: DVE tensor_scalar enters 2-port perf mode
        # and collides with SWDGE descriptor generation on GpSimd
        nmean = pool.tile([P, c.grp], FP32, tag="ln_nmean")
        nc.scalar.activation(
            nmean[:, 0:nb], msum[:, 0:nb], AF.Identity, scale=-1.0 / c.feats
        )
        sq = pool.tile([P, c.grp, P], FP32, tag="ln_sq")
        for b in range(nb):
            nc.scalar.activation(
                sq[:, b],
                x32[:, b],
                AF.Square,
                bias=nmean[:, b : b + 1],
            )
        var = pool.tile([P, c.grp], FP32, tag="ln_var")
        nc.vector.tensor_reduce(
            var[:, 0:nb], sq[:, 0:nb, :], axis=mybir.AxisListType.X, op=OP.add
        )
        rstd = pool.tile([P, c.grp], FP32, tag="ln_rstd")
        nc.scalar.activation(
            rstd[:, 0:nb],
            var[:, 0:nb],
            AF.Identity,
            scale=1.0 / c.feats,
            bias=epsc[:, 1:2],
        )
        nc.vector.reciprocal(rstd[:, 0:nb], rstd[:, 0:nb])
        nc.scalar.sqrt(rstd[:, 0:nb], rstd[:, 0:nb])
        nmr = pool.tile([P, c.grp], FP32, tag="ln_nmr")
        nc.vector.tensor_tensor(
            nmr[:, 0:nb], nmean[:, 0:nb], rstd[:, 0:nb], op=OP.mult
        )
        normed = pool.tile([P, c.grp, P], FP32, tag="ln_normed")
        for b in range(nb):
            nc.scalar.activation(
                normed[:, b],
                x32[:, b],
                AF.Identity,
                scale=rstd[:, b : b + 1],
                bias=nmr[:, b : b + 1],
            )
        out = pool.tile([P, c.grp, P], out_dtype, tag="ln_out" + str(out_dtype))
        nc.vector.tensor_tensor(
            out[:, 0:nb],
            normed[:, 0:nb],
            grep[:].rearrange("p (o f) -> p o f", o=1).to_broadcast([P, nb, P]),
            op=OP.mult,
        )
        nc.vector.tensor_tensor(
            out[:, 0:nb],
            out[:, 0:nb],
            brep[:].rearrange("p (o f) -> p o f", o=1).to_broadcast([P, nb, P]),
            op=OP.add,
        )
        return out

    with (
        tc.tile_pool(name="eg_wv", bufs=2) as eg_wv,
        tc.tile_pool(name="eg_ps", bufs=3, space="PSUM") as eg_ps,
        tc.tile_pool(name="eg_ftps", bufs=c.grp + 1, space="PSUM") as eg_ftps,
        tc.tile_pool(name="ep", bufs=2) as ep,
        tc.tile_pool(name="ep_ps", bufs=1, space="PSUM") as ep_ps,
        tc.tile_pool(name="ep_h1ps", bufs=1, space="PSUM") as ep_h1ps,
    ):
        for gi, g in enumerate(groups):
            L, L_lo, L_hi = g["L"], g["L_lo"], g["L_hi"]
            nsub = L // P
            kvE, dflR = kvEs.pop(gi), dflRs.pop(gi)
            if L_hi:
                gather_chunked(
                    lambda o, m: kvE[:, (L_lo + o) // P : (L_lo + o + m) // P, :],
                    kv_hi_t[:],
                    kvhi,
                    ohi_offs[gi],
                    L_hi,
                )

            subs = g["subs"]
            g0 = g["gsub0"]
            # one-hot dst selectors for the whole group (one DVE op each):
            # S[e, dst] from per-partition dloc, S^T[dst, e] from replicated dloc
            Sw = eg_wv.tile([P, nsub_max, P], BF16, tag="Sw")
            nc.vector.tensor_tensor(
                Sw[:, 0:nsub],
                dloc[:, g0 : g0 + nsub]
                .rearrange("p (w o) -> p w o", o=1)
                .to_broadcast([P, nsub, P]),
                iota[:].rearrange("p (o f) -> p o f", o=1).to_broadcast(
                    [P, nsub, P]
                ),
                op=OP.is_equal,
            )
            # S^T in place over the replicated dloc values
            nc.vector.tensor_tensor(
                dflR[:, 0:nsub],
                dflR[:, 0:nsub],
                iotac[:]
                .rearrange("p (w o) -> p w o", o=1)
                .to_broadcast([P, nsub, P]),
                op=OP.is_equal,
            )
            # per-wave: qE = S^T.T @ q_blk (PSUM), TT = kE * qE
            # TT and Mt share one [*, 136] tile: TT lives in cols 0:128 until
            # the head-reduce, then the same cols are overwritten with v*exp
            TM = eg_wv.tile([P, nsub_max, 136], BF16, tag="TM")
            for w0 in range(0, len(subs), c.wave):
                wsubs = subs[w0 : w0 + c.wave]
                wl = len(wsubs)
                qeps = eg_ps.tile([P, c.wave * P], FP32, tag="b32", name="qeps")
                for i, s in enumerate(wsubs):
                    nc.tensor.matmul(
                        qeps[:, i * P : (i + 1) * P],
                        dflR[:, w0 + i, :],
                        qtab[:, s["block"], :],
                        start=True,
                        stop=True,
                    )
                nc.vector.tensor_tensor(
                    TM[:, w0 : w0 + wl, 0:128],
                    kvE[:, w0 : w0 + wl, 0 : c.feats],
                    qeps[:, 0 : wl * P].rearrange("p (w f) -> p w f", f=P),
                    op=OP.mult,
                )
            # group-level: head-reduce, exp, weighted messages
            sc = eg_wv.tile([P, nsub_max, c.heads], FP32, tag="sc")
            nc.vector.tensor_reduce(
                sc[:, 0:nsub],
                TM[:, 0:nsub, 0:128].rearrange("p w (h d) -> p w h d", d=c.dhead),
                axis=mybir.AxisListType.X,
                op=OP.add,
            )
            nc.scalar.activation(
                TM[:, 0:nsub, 128:136], sc[:, 0:nsub], AF.Exp, scale=scale
            )
            nc.vector.tensor_tensor(
                TM[:, 0:nsub, 0:128].rearrange("p w (h d) -> p w h d", d=c.dhead),
                kvE[:, 0:nsub, c.feats : 2 * c.feats].rearrange(
                    "p w (h d) -> p w h d", d=c.dhead
                ),
                TM[:, 0:nsub, 128:136]
                .rearrange("p w (h o) -> p w h o", o=1)
                .to_broadcast([P, nsub, c.heads, c.dhead]),
                op=OP.mult,
            )
            ftps = {}
            for idx, s in enumerate(subs):
                b = s["block"]
                if s["first"]:
                    ftps[b] = eg_ftps.tile([P, 136], FP32, tag="ftps", name="ftps")
                nc.tensor.matmul(
                    ftps[b][:],
                    Sw[:, idx, :],
                    TM[:, idx, :],
                    start=s["first"],
                    stop=s["last"],
                    skip_group_check=True,
                )
                if s["last"]:
                    # ACT copy: a DVE copy could enter 2-port mode and block
                    # the concurrent SWDGE gather emission on GpSimd
                    nc.scalar.copy(ftden_sb[:, b, :], ftps[b][:])
                    del ftps[b]

            # ---------- epilogue for this group ----------
            bs = g["bs"]
            nb = len(bs)
            b0 = bs[0]
            f32 = ep.tile([P, c.grp, P], FP32, tag="f32")
            nc.sync.dma_start(
                f32[:, 0:nb, :],
                t["feat32_loc"][:]
                .rearrange("(s p) f -> p s f", p=P)[:, b0 : b0 + nb, :],
            )
            r = ep.tile([P, c.grp, c.heads], FP32, tag="recip")
            nc.scalar.activation(
                r[:, 0:nb], ftden_sb[:, b0 : b0 + nb, 128:136], AF.Identity,
                bias=epsc[:, 0:1],
            )
            nc.vector.reciprocal(r[:, 0:nb], r[:, 0:nb])
            rst = ep.tile([P, c.grp, P], FP32, tag="rst")
            nc.vector.tensor_tensor(
                rst[:, 0:nb],
                ftden_sb[:, b0 : b0 + nb, 0:128].rearrange(
                    "p s (h d) -> p s h d", d=c.dhead
                ),
                r[:, 0:nb].rearrange("p s (h o) -> p s h o", o=1).to_broadcast(
                    [P, nb, c.heads, c.dhead]
                ),
                op=OP.mult,
            )
            nc.vector.tensor_tensor(
                rst[:, 0:nb], rst[:, 0:nb], f32[:, 0:nb, :], op=OP.add
            )
            ln1 = layernorm(ep, rst, nb)
            ln1b = ep.tile([P, c.grp, P], BF16, tag="ln1b")
            nc.scalar.copy(ln1b[:, 0:nb], ln1[:, 0:nb])
            # transpose ln1 -> feat-major for FFN (rT and ffps share one bank):
            # bf16 transposes in cols 0:grp*P, fp32 ffps behind via bitcast
            ep2 = ep_ps.tile([P, 4 * c.grp * P], BF16, tag="ep2")
            for b in range(nb):
                nc.tensor.transpose(
                    ep2[:, b * P : (b + 1) * P], ln1b[:, b, :], ident[:]
                )
            rT = ep.tile([P, c.grp * P], BF16, tag="rT")
            nc.scalar.copy(rT[:, 0 : nb * P], ep2[:, 0 : nb * P])
            # H1 = W1.T @ rT  (feat-major, nh slices) ; prelu via W2a/W2b trick
            ff0 = 2 * c.grp * P  # ffps fp32 region starts here (in bf16 cols)
            for h in range(nh):
                h1ps = ep_h1ps.tile([P, c.grp * P], FP32, tag="h1ps")
                nc.tensor.matmul(
                    h1ps[:, 0 : nb * P],
                    w1[:, h * P : (h + 1) * P],
                    rT[:, 0 : nb * P],
                    start=True,
                    stop=True,
                )
                h1sb = ep.tile([P, c.grp * P], BF16, tag="h1sb")
                nc.scalar.activation(
                    h1sb[:, 0 : nb * P],
                    h1ps[:, 0 : nb * P],
                    AF.Identity,
                    bias=b1t[:, h : h + 1],
                )
                habs = ep.tile([P, c.grp * P], BF16, tag="habs")
                nc.scalar.activation(
                    habs[:, 0 : nb * P],
                    h1ps[:, 0 : nb * P],
                    AF.Abs,
                    bias=b1t[:, h : h + 1],
                )
                for b in range(nb):
                    ffb = ep2[:, ff0 + 2 * b * P : ff0 + 2 * (b + 1) * P].bitcast(
                        FP32
                    )
                    nc.tensor.matmul(
                        ffb,
                        h1sb[:, b * P : (b + 1) * P],
                        w2a[:, h, :],
                        start=(h == 0 and b == 0),
                        stop=False,
                        skip_group_check=True,
                    )
                    nc.tensor.matmul(
                        ffb,
                        habs[:, b * P : (b + 1) * P],
                        w2b[:, h, :],
                        start=False,
                        stop=(h == nh - 1),
                        skip_group_check=True,
                    )
            rst2 = ep.tile([P, c.grp, P], FP32, tag="rst2")
            nc.vector.tensor_tensor(
                rst2[:, 0:nb],
                ep2[:, ff0 : ff0 + 2 * nb * P]
                .bitcast(FP32)
                .rearrange("p (s f) -> p s f", f=P),
                ln1[:, 0:nb],
                op=OP.add,
            )
            nc.vector.tensor_tensor(
                rst2[:, 0:nb],
                rst2[:, 0:nb],
                b2rep[:].rearrange("p (o f) -> p o f", o=1).to_broadcast([P, nb, P]),
                op=OP.add,
            )
            ln2 = layernorm(ep, rst2, nb)
            nc.sync.dma_start(
                t["out"][:].rearrange("(s p) f -> p s f", p=P)[:, b0 : b0 + nb, :],
                ln2[:, 0:nb],
            )
            if gi + PREF < len(groups):
                issue_lo(gi + PREF)


def _build(meta, cfg: GATCfg):
    c = cfg
    nc = bacc.Bacc("TRN2", target_bir_lowering=False, debug=False, num_devices=c.n_cores)
    t = {}

    def inp(name, shape, dtype):
        t[name] = nc.dram_tensor(name, shape, dtype, kind="ExternalInput").ap()

    inp("feat16T", [c.feats, c.npad], BF16)
    inp("feat16T_loc", [c.feats, c.local_pad], BF16)
    inp("feat32_loc", [c.local_pad, c.feats], FP32)
    inp("wkv", [c.feats, 2 * c.feats], BF16)
    inp("wq", [c.feats, c.feats], BF16)
    inp("w1", [c.feats, c.dff], BF16)
    inp("w2a", [P, c.dff // P, c.feats], BF16)
    inp("w2b", [P, c.dff // P, c.feats], BF16)
    inp("b1t", [P, c.dff // P], FP32)
    inp("b2rep", [P, c.feats], FP32)
    inp("grep", [P, c.feats], FP32)
    inp("brep", [P, c.feats], FP32)
    inp("iota_row", [P, P], BF16)
    inp("iota_col", [P, 1], BF16)
    inp("ones1", [1, P], BF16)
    inp("epsc", [P, 2], FP32)
    inp("zero_idx", [P, 1], I16)
    inp("ident", [P, P], BF16)
    inp("kv_idx_lo", [P, max(meta["tot_lo"] // 16, 1)], I16)
    inp("kv_idx_hi", [P, max(meta["tot_hi"] // 16, 1)], I16)
    inp("dloc_all", [P, meta["tot_subs"]], BF16)
    inp("dloc_flat", [1, meta["tot_subs"] * P], BF16)
    t["out"] = nc.dram_tensor(
        "out", [c.local_pad, c.feats], FP32, kind="ExternalOutput"
    ).ap()

    with tile.TileContext(nc) as tc:
        _emit(tc, t, meta, cfg)
    nc.compile()
    return nc


def _in_maps(meta, streams, shared, cfg: GATCfg):
    maps = []
    for ci in range(cfg.n_cores):
        m = dict(shared)
        st = streams[ci]
        m["feat16T_loc"] = st["feat16T_loc"]
        m["feat32_loc"] = st["feat32_loc"]
        m["kv_idx_lo"] = (
            st["kv_idx_lo"] if meta["tot_lo"] else np.zeros((P, 1), np.int16)
        )
        m["kv_idx_hi"] = (
            st["kv_idx_hi"] if meta["tot_hi"] else np.zeros((P, 1), np.int16)
        )
        m["dloc_all"] = st["dloc_all"]
        m["dloc_flat"] = st["dloc_flat"]
        maps.append(m)
    return maps


_CACHE = {}


def kernel(**inputs) -> np.ndarray:
    cfg = GATCfg()
    meta, streams, shared = _prep(inputs, cfg)
    key = "real"
    if key not in _CACHE:
        _CACHE[key] = _build(meta, cfg)
    nc = _CACHE[key]
    maps = _in_maps(meta, streams, shared, cfg)
    res = run_bass_kernel_spmd(nc, maps, core_ids=list(range(cfg.n_cores)))
    out = np.empty((cfg.n_nodes, cfg.feats), np.float32)
    for ci in range(cfg.n_cores):
        out[ci * cfg.npc : (ci + 1) * cfg.npc] = res.results[ci]["out"][: cfg.npc]
    return out


# revision 77
# speedup vs baseline: 1.1262x; 1.0382x over previous
"""GAT message-passing layer on 8 Trainium2 NeuronCores (Bass/Tile).

Strategy (matches the sharding hint): nodes are partitioned across the 8
cores; each edge is owned by the core that owns its destination node, so the
segment softmax and the weighted scatter-sum stay core-local.

v2 layout (SWDGE-minimized):
  * phase 1 builds the replicated k|v projection table in DRAM from a
    host-pretransposed feature matrix (plain contiguous DMA loads, no
    DMA-transpose), and the local q table stays resident in SBUF.
  * the edge sweep fetches k and v together with a single non-transposed
    512B-row `dma_gather` per edge (one SWDGE descriptor stream instead of
    three, no xbar-transpose mode), all tensors edge-major.
  * per-edge q rows are never gathered: a one-hot dst-selector S is generated
    on-chip (compare dloc against an iota row), transposed on the PE, and
    used to expand the block's q rows with a matmul.  S also performs the
    numerator/denominator scatter-sum in one 136-column matmul per subchunk.
  * the epilogue (divide, residual, LN, FFN with PReLU folded into two
    weight matrices, LN) is emitted per group so it overlaps the gather
    stream of later groups.
"""

import sys

sys.path.insert(0, "/opt/trn_rl_repo")

import math
import os
from contextlib import ExitStack
from dataclasses import dataclass

import numpy as np
import ml_dtypes

import concourse.bass as bass
import concourse.bacc as bacc
import concourse.mybir as mybir
import concourse.tile as tile
from concourse._compat import with_exitstack
from concourse.bass_utils import run_bass_kernel_spmd
from concourse.library_config import mlp as mlp_lib

bf16 = ml_dtypes.bfloat16
P = 128
AF = mybir.ActivationFunctionType
OP = mybir.AluOpType
FP32 = mybir.dt.float32
BF16 = mybir.dt.bfloat16
I16 = mybir.dt.int16


@dataclass
class GATCfg:
    n_nodes: int = 50000
    n_edges: int = 640000
    feats: int = 128
    heads: int = 8
    dhead: int = 16
    dff: int = 512
    n_cores: int = 8
    grp: int = 2  # dst blocks per gather group
    wave: int = 4  # 128-edge subchunks per score/message wave
    pblk: int = 16  # feature blocks per phase-1 chunk
    debug: bool = False

    @property
    def npc(self):  # nodes per core
        return self.n_nodes // self.n_cores

    @property
    def nblk(self):  # local 128-node blocks per core
        return (self.npc + P - 1) // P

    @property
    def local_pad(self):
        return self.nblk * P

    @property
    def npad(self):  # padded global node count (k/v table rows)
        return ((self.n_nodes + P - 1) // P) * P

    @property
    def half(self):  # int16 index split point (block-aligned row offset base)
        h = ((self.npad // 2 + P - 1) // P) * P
        assert h < 32768 and (self.npad - h) <= 32768
        return h

    @property
    def ngrp(self):
        return (self.nblk + self.grp - 1) // self.grp


def _wrap16(idx):
    """int16 index list -> [128, n/16] SWDGE layout (16-wrap, replicated x8)."""
    idx = np.asarray(idx, np.int16)
    n = len(idx)
    assert n % 16 == 0
    return np.tile(idx.reshape(n // 16, 16).T, (8, 1)).copy()


def _prep(inputs, cfg: GATCfg):
    """Host-side graph partitioning / padding / index construction."""
    c = cfg
    feat = np.asarray(inputs["feat"], np.float32)
    src = np.asarray(inputs["src"], np.int64)
    dst = np.asarray(inputs["dst"], np.int64)

    feat_pad = np.zeros((c.npad, c.feats), np.float32)
    feat_pad[: c.n_nodes] = feat
    feat16T = np.ascontiguousarray(feat_pad.T.astype(bf16))  # [F, npad]

    # ---- per (core, block, half) edge lists ----
    core_of = dst // c.npc
    per_core = []
    for ci in range(c.n_cores):
        sel = np.nonzero(core_of == ci)[0]
        dloc = dst[sel] - ci * c.npc
        blk = dloc // P
        half = (src[sel] >= c.half).astype(np.int64)
        order = np.lexsort((dloc, half, blk))
        sel, dloc, blk, half = sel[order], dloc[order], blk[order], half[order]
        lists = {}
        for b in range(c.nblk):
            for h in range(2):
                m = (blk == b) & (half == h)
                lists[(b, h)] = (src[sel[m]], dloc[m])
        per_core.append(lists)

    # uniform sub-chunk counts across cores
    n_sub = np.zeros((c.nblk, 2), np.int64)
    for b in range(c.nblk):
        for h in range(2):
            mx = max(len(per_core[ci][(b, h)][0]) for ci in range(c.n_cores))
            n_sub[b, h] = (mx + P - 1) // P

    # ---- group structure (shared across cores) ----
    groups = []
    gsub0 = 0
    for g in range(c.ngrp):
        bs = list(range(g * c.grp, min((g + 1) * c.grp, c.nblk)))
        L_lo = int(sum(n_sub[b, 0] for b in bs)) * P
        L_hi = int(sum(n_sub[b, 1] for b in bs)) * P
        subs = []
        tot_per_block = {b: int(n_sub[b, 0] + n_sub[b, 1]) for b in bs}
        seen = {b: 0 for b in bs}
        s_idx = 0
        for h in range(2):
            for b in bs:
                for _ in range(int(n_sub[b, h])):
                    seen[b] += 1
                    subs.append(
                        dict(
                            block=b,
                            first=seen[b] == 1,
                            last=seen[b] == tot_per_block[b],
                            gsub=gsub0 + s_idx,
                        )
                    )
                    s_idx += 1
        groups.append(
            dict(bs=bs, L_lo=L_lo, L_hi=L_hi, L=L_lo + L_hi, subs=subs, gsub0=gsub0)
        )
        gsub0 += s_idx

    tot_subs = gsub0
    tot_lo = sum(g["L_lo"] for g in groups)
    tot_hi = sum(g["L_hi"] for g in groups)

    meta = dict(groups=groups, tot_subs=tot_subs, tot_lo=tot_lo, tot_hi=tot_hi)

    # ---- per-core streams ----
    per_core_streams = []
    for ci in range(c.n_cores):
        kv_lo = np.zeros(tot_lo, np.int16)
        kv_hi = np.zeros(tot_hi, np.int16)
        dloc_all = np.full((P, tot_subs), -1.0, np.float32)
        dloc_flat = np.full((1, tot_subs * P), -1.0, np.float32)
        olo = ohi = 0
        for g in groups:
            s_idx = 0
            for h in range(2):
                for b in g["bs"]:
                    s_arr, d_arr = per_core[ci][(b, h)]
                    nsub = int(n_sub[b, h])
                    npadded = nsub * P
                    rel = np.zeros(npadded, np.int16)
                    rel[: len(s_arr)] = (s_arr - (c.half if h else 0)).astype(
                        np.int16
                    )
                    if h == 0:
                        kv_lo[olo : olo + npadded] = rel
                        olo += npadded
                    else:
                        kv_hi[ohi : ohi + npadded] = rel
                        ohi += npadded
                    dl = np.full(npadded, -1.0, np.float32)
                    dl[: len(d_arr)] = (d_arr - b * P).astype(np.float32)
                    g0 = g["gsub0"] + s_idx
                    dloc_all[:, g0 : g0 + nsub] = dl.reshape(nsub, P).T
                    dloc_flat[0, g0 * P : g0 * P + npadded] = dl
                    s_idx += nsub
        feat32_loc = np.zeros((c.local_pad, c.feats), np.float32)
        feat32_loc[: c.npc] = feat[ci * c.npc : (ci + 1) * c.npc]
        feat16T_loc = np.ascontiguousarray(feat32_loc.T.astype(bf16))
        per_core_streams.append(
            dict(
                kv_idx_lo=_wrap16(kv_lo),
                kv_idx_hi=_wrap16(kv_hi),
                dloc_all=dloc_all.astype(bf16),
                dloc_flat=dloc_flat.astype(bf16),
                feat32_loc=feat32_loc,
                feat16T_loc=feat16T_loc,
            )
        )

    # ---- shared weight/constant tensors ----
    W1 = np.asarray(inputs["W1"], np.float32)
    W2 = np.asarray(inputs["W2"], np.float32)
    a = np.asarray(inputs["prelu_a"], np.float32)
    # prelu(x) = max(x,0) + a*min(x,0) = ((1+a)/2)*x + ((1-a)/2)*|x|
    nh = c.dff // P
    W2a = (
        (((1.0 + a) / 2.0)[:, None] * W2)
        .reshape(nh, P, c.feats)
        .transpose(1, 0, 2)
        .astype(bf16)
    )
    W2b = (
        (((1.0 - a) / 2.0)[:, None] * W2)
        .reshape(nh, P, c.feats)
        .transpose(1, 0, 2)
        .astype(bf16)
    )
    wk = np.asarray(inputs["Wk"], np.float32).astype(bf16)
    wv = np.asarray(inputs["Wv"], np.float32).astype(bf16)
    wkv = np.ascontiguousarray(np.hstack([wk, wv]))  # [F, 2F]
    iota_row = np.tile(np.arange(P, dtype=np.float32)[None, :], (P, 1)).astype(bf16)
    iota_col = np.arange(P, dtype=np.float32)[:, None].astype(bf16)
    ones1 = np.ones((1, P), np.float32).astype(bf16)
    shared = dict(
        feat16T=feat16T,
        wkv=wkv,
        wq=np.asarray(inputs["Wq"], np.float32).astype(bf16),
        w1=W1.astype(bf16),
        w2a=W2a,
        w2b=W2b,
        b1t=np.ascontiguousarray(
            np.asarray(inputs["b1"], np.float32).reshape(nh, P).T
        ),
        b2rep=np.tile(np.asarray(inputs["b2"], np.float32)[None, :], (P, 1)),
        grep=np.tile(np.asarray(inputs["ln1_g"], np.float32)[None, :], (P, 1)),
        brep=np.tile(np.asarray(inputs["ln1_b"], np.float32)[None, :], (P, 1)),
        iota_row=iota_row,
        iota_col=iota_col,
        ones1=ones1,
        epsc=np.tile(np.array([[1e-30, 1e-5]], np.float32), (P, 1)),
        zero_idx=np.zeros((P, 1), np.int16),
        ident=np.eye(P, dtype=np.float32).astype(bf16),
    )
    return meta, per_core_streams, shared


@with_exitstack
def _emit(ctx: ExitStack, tc: tile.TileContext, t, meta, cfg: GATCfg):
    """Emit the per-core program. `t` maps tensor name -> DRAM AP."""
    c = cfg
    nc = tc.nc
    groups = meta["groups"]
    nh = c.dff // P
    scale = 1.0 / math.sqrt(c.heads * c.dhead)

    with tc.tile_critical():
        nc.gpsimd.load_library(mlp_lib)

    # ---------- persistent pool: constants, indices, ft2 storage ----------
    keep = ctx.enter_context(tc.tile_pool(name="keep", bufs=1))

    def load_const(name, shape, dtype):
        tl = keep.tile(shape, dtype, tag=name)
        nc.sync.dma_start(tl[:], t[name][:])
        return tl

    wkv = load_const("wkv", [P, 2 * c.feats], BF16)
    wq = load_const("wq", [P, P], BF16)
    w1 = load_const("w1", [P, c.dff], BF16)
    w2a = load_const("w2a", [P, nh, c.feats], BF16)
    w2b = load_const("w2b", [P, nh, c.feats], BF16)
    b1t = load_const("b1t", [P, nh], FP32)
    b2rep = load_const("b2rep", [P, P], FP32)
    grep = load_const("grep", [P, P], FP32)
    brep = load_const("brep", [P, P], FP32)
    iota = load_const("iota_row", [P, P], BF16)
    iotac = load_const("iota_col", [P, 1], BF16)
    ones1 = load_const("ones1", [1, P], BF16)
    epsc = load_const("epsc", [P, 2], FP32)
    ident = load_const("ident", [P, P], BF16)
    kvlo = load_const("kv_idx_lo", [P, max(meta["tot_lo"] // 16, 1)], I16)
    kvhi = load_const("kv_idx_hi", [P, max(meta["tot_hi"] // 16, 1)], I16)
    dloc = load_const("dloc_all", [P, meta["tot_subs"]], BF16)

    ftden_sb = keep.tile([P, c.nblk, 136], FP32, tag="ftden_sb")
    qtab = keep.tile([P, c.nblk, c.feats], BF16, tag="qtab")
    zidx = load_const("zero_idx", [P, 1], I16)

    # dummy gather: absorbs the auto-inserted library unload/reload barrier at
    # t~0 so the first real gather is gated only by the kv-lo table writes
    gscratch = keep.tile([P, 1, 2 * c.feats], BF16, tag="gscratch")
    nc.gpsimd.dma_gather(
        gscratch[:],
        t["wkv"][:],
        zidx[:, 0:1],
        16,
        16,
        2 * c.feats,
        transpose=False,
        single_packet=False,
    )

    dram = ctx.enter_context(tc.tile_pool(name="dram", bufs=1, space="DRAM"))
    # two half tables: the lo-half gathers can start while hi is still built
    kv_lo_t = dram.tile([c.half, 2 * c.feats], BF16)
    kv_hi_t = dram.tile([c.npad - c.half, 2 * c.feats], BF16)

    # ---------- phase 1: projection tables (plain loads, batched writes) ----
    # SBUF pools stay allocated for the whole program: closing them would let
    # the edge pools reuse their memory, and that anti-dependency forces the
    # first gather to wait for ALL of phase 1 instead of just the lo table.
    nchunk = c.npad // P
    hblk = c.half // P
    prj_ft = ctx.enter_context(tc.tile_pool(name="prj_ft", bufs=2))
    prj_ftl = ctx.enter_context(tc.tile_pool(name="prj_ftl", bufs=1))
    prj_st = ctx.enter_context(tc.tile_pool(name="prj_st", bufs=2))

    GCHUNK = 1 << 30

    def gather_chunked(out_fn, in_ap, idx_tile, idx_off, n):
        for o in range(0, n, GCHUNK):
            m = min(GCHUNK, n - o)
            nc.gpsimd.dma_gather(
                out_fn(o, m),
                in_ap,
                idx_tile[:, (idx_off + o) // 16 : (idx_off + o + m) // 16],
                m,
                m,
                2 * c.feats,
                transpose=False,
                single_packet=False,
            )

    nsub_max = max(g["L"] for g in groups) // P
    olo_offs, ohi_offs = [], []
    olo = ohi = 0
    for g in groups:
        olo_offs.append(olo)
        ohi_offs.append(ohi)
        olo += g["L_lo"]
        ohi += g["L_hi"]

    # kvE/dflR pools live outside the phase-1 PSUM scope so the first few
    # groups' lo-half gathers can be issued between the lo and hi table builds
    eg_kv = ctx.enter_context(tc.tile_pool(name="eg_kv", bufs=3))
    eg_dfl = ctx.enter_context(tc.tile_pool(name="eg_dfl", bufs=3))
    PREF = 3  # lo-gather prefetch depth (= eg_kv bufs)
    kvEs, dflRs = {}, {}

    def issue_lo(gi):
        g = groups[gi]
        nsub = g["L"] // P
        kvE = eg_kv.tile([P, nsub_max, 2 * c.feats], BF16, tag="kvE")
        dflR = eg_dfl.tile([P, nsub_max, P], BF16, tag="dflR")
        nc.sync.dma_start(
            dflR[:, 0:nsub, :].rearrange("p s f -> p (s f)"),
            t["dloc_flat"][
                :, g["gsub0"] * P : g["gsub0"] * P + g["L"]
            ].to_broadcast([P, g["L"]]),
        )
        if g["L_lo"]:
            gather_chunked(
                lambda o, m: kvE[:, o // P : (o + m) // P, :],
                kv_lo_t[:],
                kvlo,
                olo_offs[gi],
                g["L_lo"],
            )
        kvEs[gi], dflRs[gi] = kvE, dflR

    with tc.tile_pool(name="prj_ps", bufs=4, space="PSUM") as prj_ps:
        def build_kv(b0, b1, table):
            for c0 in range(b0, b1, c.pblk):
                cb = min(c.pblk, b1 - c0)
                ft = prj_ft.tile([P, c.pblk * P], BF16, tag="ft")
                # loads ride the scalar HWDGE ring (drained before the edge
                # phase needs ACT); sync then carries only the table writes
                nc.scalar.dma_start(
                    ft[:, 0 : cb * P], t["feat16T"][:, c0 * P : (c0 + cb) * P]
                )
                st = prj_st.tile([P, c.pblk, 2 * c.feats], BF16, tag="st")
                for i in range(cb):
                    ps = prj_ps.tile([P, 2 * c.feats], FP32, tag="kvps")
                    nc.tensor.matmul(
                        ps[:],
                        ft[:, i * P : (i + 1) * P],
                        wkv[:],
                        start=True,
                        stop=True,
                    )
                    # alternate cast engine so neither serializes phase 1
                    if i % 2:
                        nc.scalar.copy(st[:, i, :], ps[:])
                    else:
                        nc.vector.tensor_copy(st[:, i, :], ps[:])
                nc.sync.dma_start(
                    table[:]
                    .rearrange("(s p) f -> p s f", p=P)[:, c0 - b0 : c0 - b0 + cb, :],
                    st[:, 0:cb, :],
                )

        build_kv(0, hblk, kv_lo_t)
        # lo-half gathers of the first groups run while the hi table is built
        for gi in range(min(PREF, len(groups))):
            issue_lo(gi)
        build_kv(hblk, nchunk, kv_hi_t)
        # local q table (needed only once the first group's compute starts)
        ftl = prj_ftl.tile([P, c.nblk * P], BF16, tag="ftl")
        nc.sync.dma_start(ftl[:], t["feat16T_loc"][:])
        for b in range(c.nblk):
            ps = prj_ps.tile([P, 2 * c.feats], FP32, tag="kvps")
            nc.tensor.matmul(
                ps[:, 0:P], ftl[:, b * P : (b + 1) * P], wq[:], start=True, stop=True
            )
            nc.any.tensor_copy(qtab[:, b, :], ps[:, 0:P])

    # ---------- phase 2+3: edge sweep with inline epilogue ----------
    def layernorm(pool, x32, nb, out_dtype=FP32):
        """x32: [P, nb, 128] fp32 SBUF tile -> normalized * g + b (new tile)."""
        msum = pool.tile([P, c.grp], FP32, tag="ln_msum")
        nc.vector.tensor_reduce(
            msum[:, 0:nb], x32[:, 0:nb, :], axis=mybir.AxisListType.X, op=OP.add
        )
        # scalar steps run on ACT: DVE tensor_scalar enters 2-port perf mode
        # and collides with SWDGE descriptor generation on GpSimd
        nmean = pool.tile([P, c.grp], FP32, tag="ln_nmean")
        nc.scalar.activation(
            nmean[:, 0:nb], msum[:, 0:nb], AF.Identity, scale=-1.0 / c.feats
        )
        sq = pool.tile([P, c.grp, P], FP32, tag="ln_sq")
        for b in range(nb):
            nc.scalar.activation(
                sq[:, b],
                x32[:, b],
                AF.Square,
                bias=nmean[:, b : b + 1],
            )
        var = pool.tile([P, c.grp], FP32, tag="ln_var")
        nc.vector.tensor_reduce(
            var[:, 0:nb], sq[:, 0:nb, :], axis=mybir.AxisListType.X, op=OP.add
        )
        rstd = pool.tile([P, c.grp], FP32, tag="ln_rstd")
        nc.scalar.activation(
            rstd[:, 0:nb],
            var[:, 0:nb],
            AF.Identity,
            scale=1.0 / c.feats,
            bias=epsc[:, 1:2],
        )
        nc.vector.reciprocal(rstd[:, 0:nb], rstd[:, 0:nb])
        nc.scalar.sqrt(rstd[:, 0:nb], rstd[:, 0:nb])
        nmr = pool.tile([P, c.grp], FP32, tag="ln_nmr")
        nc.vector.tensor_tensor(
            nmr[:, 0:nb], nmean[:, 0:nb], rstd[:, 0:nb], op=OP.mult
        )
        normed = pool.tile([P, c.grp, P], FP32, tag="ln_normed")
        for b in range(nb):
            nc.scalar.activation(
                normed[:, b],
                x32[:, b],
                AF.Identity,
                scale=rstd[:, b : b + 1],
                bias=nmr[:, b : b + 1],
            )
        out = pool.tile([P, c.grp, P], out_dtype, tag="ln_out" + str(out_dtype))
        nc.vector.tensor_tensor(
            out[:, 0:nb],
            normed[:, 0:nb],
            grep[:].rearrange("p (o f) -> p o f", o=1).to_broadcast([P, nb, P]),
            op=OP.mult,
        )
        nc.vector.tensor_tensor(
            out[:, 0:nb],
            out[:, 0:nb],
            brep[:].rearrange("p (o f) -> p o f", o=1).to_broadcast([P, nb, P]),
            op=OP.add,
        )
        return out

    with (
        tc.tile_pool(name="eg_wv", bufs=2) as eg_wv,
        tc.tile_pool(name="eg_ps", bufs=3, space="PSUM") as eg_ps,
        tc.tile_pool(name="eg_ftps", bufs=c.grp + 1, space="PSUM") as eg_ftps,
        tc.tile_pool(name="ep", bufs=2) as ep,
        tc.tile_pool(name="ep_ps", bufs=1, space="PSUM") as ep_ps,
        tc.tile_pool(name="ep_h1ps", bufs=1, space="PSUM") as ep_h1ps,
    ):
        for gi, g in enumerate(groups):
            L, L_lo, L_hi = g["L"], g["L_lo"], g["L_hi"]
            nsub = L // P
            kvE, dflR = kvEs.pop(gi), dflRs.pop(gi)
            if L_hi:
                gather_chunked(
                    lambda o, m: kvE[:, (L_lo + o) // P : (L_lo + o + m) // P, :],
                    kv_hi_t[:],
                    kvhi,
                    ohi_offs[gi],
                    L_hi,
                )

            subs = g["subs"]
            g0 = g["gsub0"]
            # one-hot dst selectors for the whole group (one DVE op each):
            # S[e, dst] from per-partition dloc, S^T[dst, e] from replicated dloc
            Sw = eg_wv.tile([P, nsub_max, P], BF16, tag="Sw")
            nc.vector.tensor_tensor(
                Sw[:, 0:nsub],
                dloc[:, g0 : g0 + nsub]
                .rearrange("p (w o) -> p w o", o=1)
                .to_broadcast([P, nsub, P]),
                iota[:].rearrange("p (o f) -> p o f", o=1).to_broadcast(
                    [P, nsub, P]
                ),
                op=OP.is_equal,
            )
            # S^T in place over the replicated dloc values
            nc.vector.tensor_tensor(
                dflR[:, 0:nsub],
                dflR[:, 0:nsub],
                iotac[:]
                .rearrange("p (w o) -> p w o", o=1)
                .to_broadcast([P, nsub, P]),
                op=OP.is_equal,
            )
            # per-wave: qE = S^T.T @ q_blk (PSUM), TT = kE * qE
            # TT and Mt share one [*, 136] tile: TT lives in cols 0:128 until
            # the head-reduce, then the same cols are overwritten with v*exp
            TM = eg_wv.tile([P, nsub_max, 136], BF16, tag="TM")
            for w0 in range(0, len(subs), c.wave):
                wsubs = subs[w0 : w0 + c.wave]
                wl = len(wsubs)
                qeps = eg_ps.tile([P, c.wave * P], FP32, tag="b32", name="qeps")
                for i, s in enumerate(wsubs):
                    nc.tensor.matmul(
                        qeps[:, i * P : (i + 1) * P],
                        dflR[:, w0 + i, :],
                        qtab[:, s["block"], :],
                        start=True,
                        stop=True,
                    )
                nc.vector.tensor_tensor(
                    TM[:, w0 : w0 + wl, 0:128],
                    kvE[:, w0 : w0 + wl, 0 : c.feats],
                    qeps[:, 0 : wl * P].rearrange("p (w f) -> p w f", f=P),
                    op=OP.mult,
                )
            # group-level: head-reduce, exp, weighted messages
            sc = eg_wv.tile([P, nsub_max, c.heads], FP32, tag="sc")
            nc.vector.tensor_reduce(
                sc[:, 0:nsub],
                TM[:, 0:nsub, 0:128].rearrange("p w (h d) -> p w h d", d=c.dhead),
                axis=mybir.AxisListType.X,
                op=OP.add,
            )
            nc.scalar.activation(
                TM[:, 0:nsub, 128:136], sc[:, 0:nsub], AF.Exp, scale=scale
            )
            nc.vector.tensor_tensor(
                TM[:, 0:nsub, 0:128].rearrange("p w (h d) -> p w h d", d=c.dhead),
                kvE[:, 0:nsub, c.feats : 2 * c.feats].rearrange(
                    "p w (h d) -> p w h d", d=c.dhead
                ),
                TM[:, 0:nsub, 128:136]
                .rearrange("p w (h o) -> p w h o", o=1)
                .to_broadcast([P, nsub, c.heads, c.dhead]),
                op=OP.mult,
            )
            ftps = {}
            for idx, s in enumerate(subs):
                b = s["block"]
                if s["first"]:
                    ftps[b] = eg_ftps.tile([P, 136], FP32, tag="ftps", name="ftps")
                nc.tensor.matmul(
                    ftps[b][:],
                    Sw[:, idx, :],
                    TM[:, idx, :],
                    start=s["first"],
                    stop=s["last"],
                    skip_group_check=True,
                )
                if s["last"]:
                    # ACT copy: a DVE copy could enter 2-port mode and block
                    # the concurrent SWDGE gather emission on GpSimd
                    nc.scalar.copy(ftden_sb[:, b, :], ftps[b][:])
                    del ftps[b]

            # ---------- epilogue for this group ----------
            bs = g["bs"]
            nb = len(bs)
            b0 = bs[0]
            f32 = ep.tile([P, c.grp, P], FP32, tag="f32")
            nc.sync.dma_start(
                f32[:, 0:nb, :],
                t["feat32_loc"][:]
                .rearrange("(s p) f -> p s f", p=P)[:, b0 : b0 + nb, :],
            )
            r = ep.tile([P, c.grp, c.heads], FP32, tag="recip")
            nc.scalar.activation(
                r[:, 0:nb], ftden_sb[:, b0 : b0 + nb, 128:136], AF.Identity,
                bias=epsc[:, 0:1],
            )
            nc.vector.reciprocal(r[:, 0:nb], r[:, 0:nb])
            rst = ep.tile([P, c.grp, P], FP32, tag="rst")
            nc.vector.tensor_tensor(
                rst[:, 0:nb],
                ftden_sb[:, b0 : b0 + nb, 0:128].rearrange(
                    "p s (h d) -> p s h d", d=c.dhead
                ),
                r[:, 0:nb].rearrange("p s (h o) -> p s h o", o=1).to_broadcast(
                    [P, nb, c.heads, c.dhead]
                ),
                op=OP.mult,
            )
            nc.vector.tensor_tensor(
                rst[:, 0:nb], rst[:, 0:nb], f32[:, 0:nb, :], op=OP.add
            )
            ln1 = layernorm(ep, rst, nb)
            ln1b = ep.tile([P, c.grp, P], BF16, tag="ln1b")
            nc.scalar.copy(ln1b[:, 0:nb], ln1[:, 0:nb])
            # transpose ln1 -> feat-major for FFN (rT and ffps share one bank):
            # bf16 transposes in cols 0:grp*P, fp32 ffps behind via bitcast
            ep2 = ep_ps.tile([P, 4 * c.grp * P], BF16, tag="ep2")
            for b in range(nb):
                nc.tensor.transpose(
                    ep2[:, b * P : (b + 1) * P], ln1b[:, b, :], ident[:]
                )
            rT = ep.tile([P, c.grp * P], BF16, tag="rT")
            nc.scalar.copy(rT[:, 0 : nb * P], ep2[:, 0 : nb * P])
            # H1 = W1.T @ rT  (feat-major, nh slices) ; prelu via W2a/W2b trick
            ff0 = 2 * c.grp * P  # ffps fp32 region starts here (in bf16 cols)
            for h in range(nh):
                h1ps = ep_h1ps.tile([P, c.grp * P], FP32, tag="h1ps")
                nc.tensor.matmul(
                    h1ps[:, 0 : nb * P],
                    w1[:, h * P : (h + 1) * P],
                    rT[:, 0 : nb * P],
                    start=True,
                    stop=True,
                )
                h1sb = ep.tile([P, c.grp * P], BF16, tag="h1sb")
                nc.scalar.activation(
                    h1sb[:, 0 : nb * P],
                    h1ps[:, 0 : nb * P],
                    AF.Identity,
                    bias=b1t[:, h : h + 1],
                )
                habs = ep.tile([P, c.grp * P], BF16, tag="habs")
                nc.scalar.activation(
                    habs[:, 0 : nb * P],
                    h1ps[:, 0 : nb * P],
                    AF.Abs,
                    bias=b1t[:, h : h + 1],
                )
                for b in range(nb):
                    ffb = ep2[:, ff0 + 2 * b * P : ff0 + 2 * (b + 1) * P].bitcast(
                        FP32
                    )
                    nc.tensor.matmul(
                        ffb,
                        h1sb[:, b * P : (b + 1) * P],
                        w2a[:, h, :],
                        start=(h == 0 and b == 0),
                        stop=False,
                        skip_group_check=True,
                    )
                    nc.tensor.matmul(
                        ffb,
                        habs[:, b * P : (b + 1) * P],
                        w2b[:, h, :],
                        start=False,
                        stop=(h == nh - 1),
                        skip_group_check=True,
                    )
            rst2 = ep.tile([P, c.grp, P], FP32, tag="rst2")
            nc.vector.tensor_tensor(
                rst2[:, 0:nb],
                ep2[:, ff0 : ff0 + 2 * nb * P]
                .bitcast(FP32)
                .rearrange("p (s f) -> p s f", f=P),
                ln1[:, 0:nb],
                op=OP.add,
            )
            nc.vector.tensor_tensor(
                rst2[:, 0:nb],
                rst2[:, 0:nb],
                b2rep[:].rearrange("p (o f) -> p o f", o=1).to_broadcast([P, nb, P]),
                op=OP.add,
            )
            ln2 = layernorm(ep, rst2, nb)
            nc.sync.dma_start(
                t["out"][:].rearrange("(s p) f -> p s f", p=P)[:, b0 : b0 + nb, :],
                ln2[:, 0:nb],
            )
            if gi + PREF < len(groups):
                issue_lo(gi + PREF)


def _build(meta, cfg: GATCfg):
    c = cfg
    nc = bacc.Bacc("TRN2", target_bir_lowering=False, debug=False, num_devices=c.n_cores)
    t = {}

    def inp(name, shape, dtype):
        t[name] = nc.dram_tensor(name, shape, dtype, kind="ExternalInput").ap()

    inp("feat16T", [c.feats, c.npad], BF16)
    inp("feat16T_loc", [c.feats, c.local_pad], BF16)
    inp("feat32_loc", [c.local_pad, c.feats], FP32)
    inp("wkv", [c.feats, 2 * c.feats], BF16)
    inp("wq", [c.feats, c.feats], BF16)
    inp("w1", [c.feats, c.dff], BF16)
    inp("w2a", [P, c.dff // P, c.feats], BF16)
    inp("w2b", [P, c.dff // P, c.feats], BF16)
    inp("b1t", [P, c.dff // P], FP32)
    inp("b2rep", [P, c.feats], FP32)
    inp("grep", [P, c.feats], FP32)
    inp("brep", [P, c.feats], FP32)
    inp("iota_row", [P, P], BF16)
    inp("iota_col", [P, 1], BF16)
    inp("ones1", [1, P], BF16)
    inp("epsc", [P, 2], FP32)
    inp("zero_idx", [P, 1], I16)
    inp("ident", [P, P], BF16)
    inp("kv_idx_lo", [P, max(meta["tot_lo"] // 16, 1)], I16)
    inp("kv_idx_hi", [P, max(meta["tot_hi"] // 16, 1)], I16)
    inp("dloc_all", [P, meta["tot_subs"]], BF16)
    inp("dloc_flat", [1, meta["tot_subs"] * P], BF16)
    t["out"] = nc.dram_tensor(
        "out", [c.local_pad, c.feats], FP32, kind="ExternalOutput"
    ).ap()

    with tile.TileContext(nc) as tc:
        _emit(tc, t, meta, cfg)
    nc.compile()
    return nc


def _in_maps(meta, streams, shared, cfg: GATCfg):
    maps = []
    for ci in range(cfg.n_cores):
        m = dict(shared)
        st = streams[ci]
        m["feat16T_loc"] = st["feat16T_loc"]
        m["feat32_loc"] = st["feat32_loc"]
        m["kv_idx_lo"] = (
            st["kv_idx_lo"] if meta["tot_lo"] else np.zeros((P, 1), np.int16)
        )
        m["kv_idx_hi"] = (
            st["kv_idx_hi"] if meta["tot_hi"] else np.zeros((P, 1), np.int16)
        )
        m["dloc_all"] = st["dloc_all"]
        m["dloc_flat"] = st["dloc_flat"]
        maps.append(m)
    return maps


_CACHE = {}


def kernel(**inputs) -> np.ndarray:
    cfg = GATCfg()
    meta, streams, shared = _prep(inputs, cfg)
    key = "real"
    if key not in _CACHE:
        _CACHE[key] = _build(meta, cfg)
    nc = _CACHE[key]
    maps = _in_maps(meta, streams, shared, cfg)
    res = run_bass_kernel_spmd(nc, maps, core_ids=list(range(cfg.n_cores)))
    out = np.empty((cfg.n_nodes, cfg.feats), np.float32)
    for ci in range(cfg.n_cores):
        out[ci * cfg.npc : (ci + 1) * cfg.npc] = res.results[ci]["out"][: cfg.npc]
    return out
